# revision 42
# baseline (speedup 1.0000x reference)
"""Trainium2 Bass kernel for nn_CustomMetalPKA_GNN (gnn_message_passing).

Distribution: node-sharded GCN message passing across 8 NeuronCores.
Each core owns a contiguous block of 1280 node rows (10 windows of 128).
Edges are assigned to the core owning their destination node, sorted by
destination, deduplicated by source within each window, and processed as
128-row gather tiles.

Per window the scatter-add is a chain of matmuls: the gathered source rows
(fp8 e4m3, x pre-scaled by 16) are the stationary operand and a
host-precomputed one-hot matrix (entries = the GCN norm dinv[s]*dinv[d]
scaled by 4, fp8, shared by both layers) is the moving operand; the PSUM
result is the *transposed* aggregate at scale 64. Self-loops use SBUF-
resident tiles (own x rows / own t2 rows) against a diagonal one-hot block,
so they cost no gather descriptors. All dense layers chain off the
transposed layout with zero PE transposes in bf16; the fp8 scales fold into
the weights (g1/64, g2*16, lp/64, b2*64):
  l1: aggT -> (W1^T/64 chunks) -> hT -> relu+bias -> t2*16 rows = h @ g2*16
      -> fp8 table2 rows -> one full fp8 AllGather per rep
  l2: agg2T(*64) -> relu(in+64*b2) -> h1 = relu(h2 @ lp + b) -> tail mask
Scatter matmuls contract two 128-edge tiles at once via fp8 DoubleRow.
Across timing reps the kernel is software-pipelined: layer 2 trails its
own rep by two, so each AllGather is covered by ~two reps of gather and
matmul work (table2/local_rows rotate over three DRAM buffers).
The tiny metal/transformer tail is reduced to an [8, 512] summary
(3 ligand-block sums + 3 prediction rows) and finished on host.
"""

import os
import sys

for _p in ("/opt/trn_rl_repo", "/root/.axon_site/_ro/trn_rl_repo"):
    if os.path.isdir(_p) and _p not in sys.path:
        sys.path.insert(0, _p)

import numpy as np

import concourse.bacc as bacc
import concourse.tile as tile
from concourse import bass, mybir
from concourse.bass_utils import run_bass_kernel_spmd

# Problem shapes (hardcoded per spec)
N = 9999
E = 160000
NODE_D = 256
HID = 512
MAX_LIG = 3
APL = N // MAX_LIG  # 3333

NCORES = 8
P = 128
WPC = 10                 # windows per core
NPC = WPC * P            # 1280 nodes per core
NPAD = NCORES * NPC      # 10240
HALF = NPC // 2

FP = mybir.dt.float32
BF = mybir.dt.bfloat16
F8 = mybir.dt.float8e4
I16 = mybir.dt.int16
NPBF = mybir.dt.np(BF)
NPF8 = mybir.dt.np(F8)
GCHUNK = 8  # gather tiles per dma_gather call (SWDGE desc ring <= 1024)
NQ = 4      # SWDGE queues

XS = 16.0   # x fp8 pre-scale
OS = 4.0    # one-hot norm fp8 pre-scale
T2S = 16.0  # t2 fp8 pre-scale (folded into g2)

_RUN_CACHE = {}


def _q8(a):
    return np.clip(np.asarray(a, np.float32), -240.0, 240.0).astype(NPF8)


# ----------------------------------------------------------------------------
# Host-side graph preprocessing (index/structure only)
# ----------------------------------------------------------------------------

def _prep(x, edge_index, pred_pos):
    src = np.asarray(edge_index[0], dtype=np.int64)
    dst = np.asarray(edge_index[1], dtype=np.int64)
    pred_pos = np.asarray(pred_pos, dtype=np.int64)

    deg = np.bincount(dst, minlength=N).astype(np.float32) + 1.0
    dinv = deg ** -0.5

    order = np.argsort(dst, kind="stable")
    s_s = src[order]
    d_s = dst[order]
    n_s = dinv[s_s] * dinv[d_s]

    # per (core, window): dedup'd source list + per-edge (slot, dstoff, norm)
    uniq = {}
    for c in range(NCORES):
        for w in range(WPC):
            lo = c * NPC + w * P
            hi = min(lo + P, N)
            if lo >= N:
                z = np.zeros(0, np.int64)
                uniq[c, w] = (z, z, z.astype(np.float32), z)
                continue
            a = np.searchsorted(d_s, lo, side="left")
            b = np.searchsorted(d_s, hi, side="left")
            us, inv = np.unique(s_s[a:b], return_inverse=True)
            uniq[c, w] = (us, inv, n_s[a:b], d_s[a:b] - lo)

    # shared tile counts (instruction stream is identical across cores)
    T = [max(1, max((len(uniq[c, w][0]) + P - 1) // P for c in range(NCORES)))
         for w in range(WPC)]
    Tsum = sum(T)                  # gather tiles
    Toh = [t + 1 for t in T]       # +1 self block per window
    Tohsum = sum(Toh)
    CTOT = 8 * Tsum                # int16 index columns

    selfw = dinv * dinv  # self-loop weight per node

    per_core = []
    for c in range(NCORES):
        gidx = np.zeros((P, CTOT), np.int16)
        ohf = np.zeros((P, Tohsum * P), np.float32)
        goff = 0   # gather tile offset
        ooff = 0   # oh column-block offset
        for w in range(WPC):
            us, inv, nrm, e_doff = uniq[c, w]
            base = c * NPC + w * P
            nreal = max(0, min(base + P, N) - base)
            # self block: diagonal of selfw * OS
            if nreal > 0:
                rr = np.arange(nreal)
                ohf[rr, ooff * P + rr] = selfw[base + rr] * OS
            # gather index columns (same node ids serve x and table2: the
            # single full AllGather keeps table2 rows in node order)
            cap = T[w] * P
            srcs = np.zeros(cap, np.int64)
            srcs[:len(us)] = us
            colbase = 8 * goff
            ii = np.arange(cap)
            gidx[ii % 16, colbase + ii // 16] = srcs.astype(np.int16)
            # one-hot: edge e -> oh[slot%128, blk(e)*P + doff]
            if len(nrm) > 0:
                cols = (ooff + 1 + inv // P) * P + e_doff
                np.add.at(ohf, (inv % P, cols), nrm * OS)
            goff += T[w]
            ooff += Toh[w]
        gidx[16:] = np.tile(gidx[:16], (7, 1))

        # tail masks [P, 8 * WPC]
        tmask = np.zeros((P, 8 * WPC), NPBF)
        for w in range(WPC):
            base = c * NPC + w * P
            nodes = base + np.arange(P)
            real = nodes < N
            for b2 in range(MAX_LIG):
                sel = real & (nodes >= b2 * APL) & (nodes < (b2 + 1) * APL)
                tmask[sel, 8 * w + b2] = 1.0
            for i in range(MAX_LIG):
                sel = nodes == pred_pos[i]
                tmask[sel, 8 * w + 3 + i] = 1.0
        per_core.append(dict(gidx=gidx, oh=_q8(ohf), tmask=tmask))

    x_pad = np.zeros((NPAD, NODE_D), NPF8)
    x_pad[:N] = _q8(np.asarray(x, np.float32) * XS)

    meta = dict(T=T, Tsum=Tsum, Toh=Toh, Tohsum=Tohsum, CTOT=CTOT)
    return meta, per_core, x_pad


def build_in_maps(inputs):
    meta, per_core, x_pad = _prep(
        np.asarray(inputs["x"], np.float32), np.asarray(inputs["edge_index"]),
        np.asarray(inputs["pred_pos"]))
    b1T = np.asarray(inputs["g1_b"], np.float32).reshape(HID // P, P).T.copy()
    b2T = (np.asarray(inputs["g2_b"], np.float32) * (T2S * OS)
           ).reshape(HID // P, P).T.copy()
    blp_rep = np.tile(np.asarray(inputs["lp_b"], np.float32)[None, :], (P, 1))
    g1 = (np.asarray(inputs["g1_w"], np.float32) / (XS * OS)).astype(NPBF)
    g2 = (np.asarray(inputs["g2_w"], np.float32) * T2S).astype(NPBF)
    lp = (np.asarray(inputs["lp_w"], np.float32) / (T2S * OS)).astype(NPBF)
    in_maps = []
    for c in range(NCORES):
        pc = per_core[c]
        in_maps.append(dict(
            x_f8=x_pad, xself=x_pad[c * NPC:(c + 1) * NPC].copy(),
            gidx=pc["gidx"], oh=pc["oh"],
            tmask=pc["tmask"], g1_w=g1, g2_w=g2, lp_w=lp,
            b1T=b1T, b2T=b2T, blp_rep=blp_rep,
        ))
    return meta, in_maps


# ----------------------------------------------------------------------------
# Device program
# ----------------------------------------------------------------------------

def _build(meta, reps=1):
    T = meta["T"]
    Tsum = meta["Tsum"]
    Toh = meta["Toh"]
    Tohsum = meta["Tohsum"]
    CTOT = meta["CTOT"]

    nc = bacc.Bacc("TRN2", target_bir_lowering=False, debug=False,
                   num_devices=NCORES, num_swdge_queues=NQ)

    # inputs
    d_x8 = nc.declare_dram_parameter("x_f8", [NPAD, NODE_D], F8, isOutput=False)
    d_xself = nc.declare_dram_parameter("xself", [NPC, NODE_D], F8, isOutput=False)
    d_gidx = nc.declare_dram_parameter("gidx", [P, CTOT], I16, isOutput=False)
    d_oh = nc.declare_dram_parameter("oh", [P, Tohsum * P], F8, isOutput=False)
    d_tmask = nc.declare_dram_parameter("tmask", [P, 8 * WPC], BF, isOutput=False)
    d_g1 = nc.declare_dram_parameter("g1_w", [NODE_D, HID], BF, isOutput=False)
    d_g2 = nc.declare_dram_parameter("g2_w", [HID, HID], BF, isOutput=False)
    d_lp = nc.declare_dram_parameter("lp_w", [HID, HID], BF, isOutput=False)
    d_b1T = nc.declare_dram_parameter("b1T", [P, HID // P], FP, isOutput=False)
    d_b2T = nc.declare_dram_parameter("b2T", [P, HID // P], FP, isOutput=False)
    d_blp = nc.declare_dram_parameter("blp_rep", [P, HID], FP, isOutput=False)
    # output
    d_tail = nc.declare_dram_parameter("out_tail", [8, HID], FP, isOutput=True)

    # internal dram (triple-buffered by rep parity so the exchange of rep r
    # never conflicts with older reps' layer-2 reads, two reps behind)
    local_rows = [nc.dram_tensor(f"local_rows{i}", [NPC, HID], F8)
                  for i in range(3)]
    table2 = [nc.dram_tensor(f"table2_{i}", [NPAD, HID], F8,
                             addr_space="Shared") for i in range(3)]

    mm = mybir.AluOpType
    act = mybir.ActivationFunctionType
    KC1 = NODE_D // P   # 2 feature chunks in layer-1 scatter
    KC2 = HID // P      # 4 chunks in layer-2 / dense

    with tile.TileContext(nc) as tc:
        with (
            tc.tile_pool(name="const", bufs=1) as cpool,
            tc.tile_pool(name="work", bufs=6) as wpool,
            tc.tile_pool(name="persist", bufs=3) as ppool,
            tc.tile_pool(name="hT", bufs=6) as hTpool,
            tc.tile_pool(name="ps_sc", bufs=3, space="PSUM") as ps_sc,
            tc.tile_pool(name="ps_mm", bufs=3, space="PSUM") as ps_mm,
            tc.tile_pool(name="ps_tail", bufs=1, space="PSUM") as ps_tail,
        ):
            # ---- constants in ----
            t_idx = cpool.tile([P, CTOT], I16)
            nc.sync.dma_start(t_idx[:], d_gidx[:])
            t_oh = cpool.tile([P, Tohsum * P], F8)
            nc.sync.dma_start(t_oh[:], d_oh[:])
            t_tmask = cpool.tile([P, 8 * WPC], BF)
            nc.sync.dma_start(t_tmask[:], d_tmask[:])
            t_g1 = cpool.tile([P, KC1, HID], BF)
            nc.sync.dma_start(t_g1[:], d_g1.rearrange("(a p) n -> p a n", p=P))
            t_g2 = cpool.tile([P, KC2, HID], BF)
            nc.sync.dma_start(t_g2[:], d_g2.rearrange("(a p) n -> p a n", p=P))
            t_lp = cpool.tile([P, KC2, HID], BF)
            nc.sync.dma_start(t_lp[:], d_lp.rearrange("(a p) n -> p a n", p=P))
            t_b1T = cpool.tile([P, HID // P], FP)
            nc.sync.dma_start(t_b1T[:], d_b1T[:])
            t_b2T = cpool.tile([P, HID // P], FP)
            nc.sync.dma_start(t_b2T[:], d_b2T[:])
            t_blp = cpool.tile([P, HID], FP)
            nc.sync.dma_start(t_blp[:], d_blp[:])

            def OH(col):
                return t_oh[:, col * P:(col + 1) * P]

            def OH2(col):
                return t_oh[:, col * P:(col + 2) * P].rearrange(
                    "p (j n) -> p j n", j=2)

            DR = mybir.MatmulPerfMode.DoubleRow

            def oh_scatter(out_ap, gt, fc, c0, Tn, first_start):
                # accumulate Tn gathered fp8 tiles against OH blocks
                # c0..c0+Tn-1, pairing tiles via fp8 DoubleRow (2 edge-tiles
                # contracted per matmul).
                t = 0
                first = first_start
                while t < Tn:
                    if t + 1 < Tn:
                        nc.tensor.matmul(
                            out_ap, gt[:, t:t + 2, fc * P:(fc + 1) * P],
                            OH2(c0 + t), start=first, stop=(t + 2 == Tn),
                            perf_mode=DR)
                        t += 2
                    else:
                        nc.tensor.matmul(
                            out_ap, gt[:, t, fc * P:(fc + 1) * P],
                            OH(c0 + t), start=first, stop=True)
                        t += 1
                    first = False

            qn = [0]

            def gathers(out_tile, table_ap, idx_tile, gcol, Tw, elem):
                # out_tile[:, t, :] = table[idx[gcol + t*128 + p]] tiles
                for t0 in range(0, Tw, GCHUNK):
                    t1 = min(t0 + GCHUNK, Tw)
                    nc.gpsimd.dma_gather(
                        out_ap=out_tile[:, t0:t1, :],
                        in_ap=table_ap,
                        idxs_ap=idx_tile[:, 8 * (gcol + t0): 8 * (gcol + t1)],
                        num_idxs=(t1 - t0) * P,
                        num_idxs_reg=(t1 - t0) * P,
                        elem_size=elem,
                        queue_num=qn[0] % NQ)
                    qn[0] += 1

            # self-row x tiles: core-local 1280 rows as [P, WPC, NODE_D]
            t_xs = cpool.tile([P, WPC, NODE_D], F8)
            nc.sync.dma_start(
                t_xs[:], d_xself.rearrange("(w p) n -> p w n", p=P))

            g1ctx = tc.tile_pool(name="g1pool", bufs=6)
            g1pool = g1ctx.__enter__()
            g2ctx = tc.tile_pool(name="g2pool", bufs=6)
            g2pool = g2ctx.__enter__()

            GOFF = np.cumsum([0] + T)     # gather tile offset per window
            OOFF = np.cumsum([0] + Toh)   # oh column-block offset per window

            def emit_l1_window(w, t2keep, lrows):
                Tw = T[w]
                goff, ooff = int(GOFF[w]), int(OOFF[w])
                g1t = g1pool.tile([P, Tw, NODE_D], F8, tag="gather1")
                gathers(g1t, d_x8[:], t_idx, goff, Tw, NODE_D)
                # aggT[k, d] = sum_e x[s_e, k] * norm_e [dst off d] (x64)
                psT = ps_sc.tile([P, NODE_D], FP, tag="psc")
                for fc in range(KC1):
                    nc.tensor.matmul(
                        psT[:, fc * P:(fc + 1) * P],
                        t_xs[:, w, fc * P:(fc + 1) * P],
                        OH(ooff), start=True, stop=False)
                    oh_scatter(psT[:, fc * P:(fc + 1) * P], g1t, fc,
                               ooff + 1, Tw, False)
                aggT = wpool.tile([P, NODE_D], BF, tag="aggT")
                nc.vector.tensor_copy(aggT[:], psT[:])
                # hT[j, d] = relu(sum_k (W1/64)[k, j] aggT[k, d] + b1)
                phT = ps_mm.tile([P, HID], FP, tag="pmm")
                for jc in range(KC2):
                    for fc in range(KC1):
                        nc.tensor.matmul(
                            phT[:, jc * P:(jc + 1) * P],
                            t_g1[:, fc, jc * P:(jc + 1) * P],
                            aggT[:, fc * P:(fc + 1) * P],
                            start=(fc == 0), stop=(fc == KC1 - 1))
                hT = hTpool.tile([P, HID], BF, tag="hT")
                for jc in range(KC2):
                    nc.scalar.activation(
                        hT[:, jc * P:(jc + 1) * P],
                        phT[:, jc * P:(jc + 1) * P],
                        act.Relu, bias=t_b1T[:, jc:jc + 1])
                # t2[d, m]*16 = sum_j h[d, j] (16*g2)[j, m] -> fp8 rows
                pt2 = ps_mm.tile([P, HID], FP, tag="pmm")
                for jc in range(KC2):
                    nc.tensor.matmul(pt2[:], hT[:, jc * P:(jc + 1) * P],
                                     t_g2[:, jc, :],
                                     start=(jc == 0), stop=(jc == KC2 - 1))
                nc.vector.tensor_copy(t2keep[:, w, :], pt2[:])
                nc.sync.dma_start(lrows[w * P:(w + 1) * P, :],
                                  t2keep[:, w, :])

            def emit_l2(st):
                t2keep = st["t2keep"]
                tab = st["table2"]
                ptail = ps_tail.tile([8, HID], FP)
                for w in range(WPC):
                    goff, ooff = int(GOFF[w]), int(OOFF[w])
                    g2t = g2pool.tile([P, T[w], HID], F8, tag="gather2")
                    gathers(g2t, tab[:], t_idx, goff, T[w], HID)
                    ps2 = ps_sc.tile([P, HID], FP, tag="psc")
                    for fc in range(KC2):
                        nc.tensor.matmul(
                            ps2[:, fc * P:(fc + 1) * P],
                            t2keep[:, w, fc * P:(fc + 1) * P],
                            OH(ooff), start=True, stop=False)
                        oh_scatter(ps2[:, fc * P:(fc + 1) * P], g2t, fc,
                                   ooff + 1, T[w], False)
                    # h2T*64 = relu(agg2*64 + 64*b2)
                    h2T = hTpool.tile([P, HID], BF, tag="hT")
                    for fc in range(KC2):
                        nc.scalar.activation(
                            h2T[:, fc * P:(fc + 1) * P],
                            ps2[:, fc * P:(fc + 1) * P],
                            act.Relu, bias=t_b2T[:, fc:fc + 1])
                    ph1 = ps_mm.tile([P, HID], FP, tag="pmm")
                    for jc in range(KC2):
                        nc.tensor.matmul(ph1[:], h2T[:, jc * P:(jc + 1) * P],
                                         t_lp[:, jc, :],
                                         start=(jc == 0), stop=(jc == KC2 - 1))
                    h1s = wpool.tile([P, HID], FP, tag="h1s")
                    nc.vector.tensor_add(h1s[:], ph1[:], t_blp[:])
                    h1 = wpool.tile([P, HID], BF, tag="h1")
                    nc.scalar.activation(h1[:], h1s[:], act.Relu)
                    nc.tensor.matmul(ptail[:], t_tmask[:, 8 * w:8 * w + 8],
                                     h1[:],
                                     start=(w == 0), stop=(w == WPC - 1))
                t_tail = wpool.tile([8, HID], FP, tag="tailout")
                nc.vector.tensor_copy(t_tail[:], ptail[:])
                nc.sync.dma_start(d_tail[:], t_tail[:])

            # Software pipeline across reps: one full AllGather per rep,
            # whose latency is hidden by ~two reps of other work (layer 2
            # trails its own rep by two). table2/local_rows rotate by rep%3.
            pending = []
            for rep_i in range(reps):
                pr = rep_i % 3
                t2keep = ppool.tile([P, WPC, HID], F8, tag="t2keep")
                st = {"t2keep": t2keep, "table2": table2[pr]}
                for w in range(WPC):
                    emit_l1_window(w, t2keep, local_rows[pr])
                nc.gpsimd.collective_compute(
                    "AllGather", mm.bypass,
                    replica_groups=[list(range(NCORES))],
                    ins=[local_rows[pr][:, :]], outs=[table2[pr][:, :]])
                pending.append(st)
                if len(pending) > 2:
                    emit_l2(pending.pop(0))
            for st in pending:
                emit_l2(st)
            g2ctx.__exit__(None, None, None)
            g1ctx.__exit__(None, None, None)

    nc.compile()
    return nc


# ----------------------------------------------------------------------------
# Host tail (metal branch + gates + 4-node TransformerConv + MLP head)
# ----------------------------------------------------------------------------

def _host_tail(tail, pred_pos, metal_id, metal_emb_table, mp_w, mp_b,
               gate_w1, gate_b1, gate_w2, gate_b2,
               tq_w, tq_b, tk_w, tk_b, tv_w, tv_b, tskip_w, tskip_b,
               pr_w1, pr_b1, pr_w2, pr_b2):
    f = np.float32
    pred_pos = np.asarray(pred_pos, np.int64)
    blocksum = tail[:3].astype(f)
    predrow = tail[3:6].astype(f)
    HEADS, HD = 8, HID // 8

    backbones = []
    for i in range(MAX_LIG):
        b = int(pred_pos[i]) // APL
        backbones.append((blocksum[b] - predrow[i]) / f(APL - 1))

    metal_node = np.maximum(
        np.asarray(metal_emb_table, f)[np.asarray(metal_id, np.int64)] @
        np.asarray(mp_w, f) + np.asarray(mp_b, f), 0)

    def tconv(hm, es, ed):
        n = hm.shape[0]
        q = (hm @ np.asarray(tq_w, f) + np.asarray(tq_b, f)).reshape(n, HEADS, HD)
        k = (hm @ np.asarray(tk_w, f) + np.asarray(tk_b, f)).reshape(n, HEADS, HD)
        v = (hm @ np.asarray(tv_w, f) + np.asarray(tv_b, f)).reshape(n, HEADS, HD)
        kj = k[es]
        vj = v[es]
        alpha = (q[ed] * kj).sum(-1) / np.sqrt(f(HD))
        amax = np.full((n, HEADS), -np.inf, f)
        np.maximum.at(amax, ed, alpha)
        ae = np.exp(alpha - amax[ed])
        den = np.zeros((n, HEADS), f)
        np.add.at(den, ed, ae)
        att = ae / den[ed]
        out = np.zeros((n, HEADS, HD), f)
        np.add.at(out, ed, vj * att[:, :, None])
        return out.reshape(n, HID) + hm @ np.asarray(tskip_w, f) + np.asarray(tskip_b, f)

    preds = []
    for n_lig in range(MAX_LIG, 0, -1):
        rows = []
        for i in range(n_lig):
            hb = backbones[i]
            g = 1.0 / (1.0 + np.exp(-(np.tanh(hb @ np.asarray(gate_w1, f) +
                                              np.asarray(gate_b1, f)) @
                                      np.asarray(gate_w2, f) +
                                      np.asarray(gate_b2, f))))
            rows.append(predrow[i] + g[0] * hb)
        hm = np.concatenate([metal_node, np.stack(rows)], 0).astype(f)
        es, ed = [], []
        for l in range(1, n_lig + 1):
            es += [0, l]
            ed += [l, 0]
        h3 = tconv(hm, np.array(es), np.array(ed))
        V = h3.mean(0)
        preds.append((V @ np.asarray(pr_w1, f) + np.asarray(pr_b1, f)) @
                     np.asarray(pr_w2, f) + np.asarray(pr_b2, f))
    return np.concatenate(preds).astype(np.float32)


# ----------------------------------------------------------------------------
# Entry point
# ----------------------------------------------------------------------------

def kernel(**inputs):
    meta, in_maps = build_in_maps(inputs)

    key = (meta["Tsum"], tuple(meta["T"]))
    nc = _RUN_CACHE.get(key)
    if nc is None:
        nc = _build(meta)
        _RUN_CACHE[key] = nc

    res = run_bass_kernel_spmd(nc, in_maps, list(range(NCORES)))
    tail = np.zeros((8, HID), np.float32)
    for c in range(NCORES):
        tail += res.results[c]["out_tail"]

    pred_pos = np.asarray(inputs["pred_pos"])
    return _host_tail(
        tail, pred_pos, inputs["metal_id"], inputs["metal_emb_table"],
        inputs["mp_w"], inputs["mp_b"],
        inputs["gate_w1"], inputs["gate_b1"], inputs["gate_w2"], inputs["gate_b2"],
        inputs["tq_w"], inputs["tq_b"], inputs["tk_w"], inputs["tk_b"],
        inputs["tv_w"], inputs["tv_b"], inputs["tskip_w"], inputs["tskip_b"],
        inputs["pr_w1"], inputs["pr_b1"], inputs["pr_w2"], inputs["pr_b2"])


# revision 43
# speedup vs baseline: 1.5364x; 1.5364x over previous
"""Trainium2 Bass kernel for nn_CustomMetalPKA_GNN (gnn_message_passing).

Distribution: node-sharded GCN message passing across 8 NeuronCores.
Each core owns a contiguous block of 1280 node rows (10 windows of 128).
Edges are assigned to the core owning their destination node, sorted by
destination, deduplicated by source within each window, and processed as
128-row gather tiles.

Per window the scatter-add is a chain of matmuls: the gathered source rows
(fp8 e4m3, x pre-scaled by 16) are the stationary operand and a
host-precomputed one-hot matrix (entries = the GCN norm dinv[s]*dinv[d]
scaled by 4, fp8, shared by both layers) is the moving operand; the PSUM
result is the *transposed* aggregate at scale 64. Self-loops use SBUF-
resident tiles (own x rows / own t2 rows) against a diagonal one-hot block,
so they cost no gather descriptors. All dense layers chain off the
transposed layout with zero PE transposes in bf16; the fp8 scales fold into
the weights (g1/64, g2*16, lp/64, b2*64):
  l1: aggT -> (W1^T/64 chunks) -> hT -> relu+bias -> t2*16 rows = h @ g2*16
      -> fp8 table2 rows -> one full fp8 AllGather per rep
  l2: agg2T(*64) -> relu(in+64*b2) -> h1 = relu(h2 @ lp + b) -> tail mask
Scatter matmuls contract two 128-edge tiles at once via fp8 DoubleRow.
Across timing reps the kernel is software-pipelined: layer 2 trails its
own rep by two, so each AllGather is covered by ~two reps of gather and
matmul work (table2/local_rows rotate over three DRAM buffers).
The tiny metal/transformer tail is reduced to an [8, 512] summary
(3 ligand-block sums + 3 prediction rows) and finished on host.
"""

import os
import sys

for _p in ("/opt/trn_rl_repo", "/root/.axon_site/_ro/trn_rl_repo"):
    if os.path.isdir(_p) and _p not in sys.path:
        sys.path.insert(0, _p)

import numpy as np

import concourse.bacc as bacc
import concourse.tile as tile
from concourse import bass, mybir
from concourse.bass_utils import run_bass_kernel_spmd

# Problem shapes (hardcoded per spec)
N = 9999
E = 160000
NODE_D = 256
HID = 512
MAX_LIG = 3
APL = N // MAX_LIG  # 3333

NCORES = 8
P = 128
WPC = 10                 # windows per core
NPC = WPC * P            # 1280 nodes per core
NPAD = NCORES * NPC      # 10240
HALF = NPC // 2

FP = mybir.dt.float32
BF = mybir.dt.bfloat16
F8 = mybir.dt.float8e4
I16 = mybir.dt.int16
NPBF = mybir.dt.np(BF)
NPF8 = mybir.dt.np(F8)
GCHUNK = 8  # gather tiles per dma_gather call (SWDGE desc ring <= 1024)
NQ = 4      # SWDGE queues

XS = 16.0   # x fp8 pre-scale
OS = 4.0    # one-hot norm fp8 pre-scale
T2S = 16.0  # t2 fp8 pre-scale (folded into g2)

_RUN_CACHE = {}


def _q8(a):
    return np.clip(np.asarray(a, np.float32), -240.0, 240.0).astype(NPF8)


# ----------------------------------------------------------------------------
# Host-side graph preprocessing (index/structure only)
# ----------------------------------------------------------------------------

def _prep(x, edge_index, pred_pos):
    src = np.asarray(edge_index[0], dtype=np.int64)
    dst = np.asarray(edge_index[1], dtype=np.int64)
    pred_pos = np.asarray(pred_pos, dtype=np.int64)

    deg = np.bincount(dst, minlength=N).astype(np.float32) + 1.0
    dinv = deg ** -0.5

    order = np.argsort(dst, kind="stable")
    s_s = src[order]
    d_s = dst[order]
    n_s = dinv[s_s] * dinv[d_s]

    # per (core, window): dedup'd source list + per-edge (slot, dstoff, norm)
    uniq = {}
    for c in range(NCORES):
        for w in range(WPC):
            lo = c * NPC + w * P
            hi = min(lo + P, N)
            if lo >= N:
                z = np.zeros(0, np.int64)
                uniq[c, w] = (z, z, z.astype(np.float32), z)
                continue
            a = np.searchsorted(d_s, lo, side="left")
            b = np.searchsorted(d_s, hi, side="left")
            us, inv = np.unique(s_s[a:b], return_inverse=True)
            uniq[c, w] = (us, inv, n_s[a:b], d_s[a:b] - lo)

    # shared tile counts (instruction stream is identical across cores)
    T = [max(1, max((len(uniq[c, w][0]) + P - 1) // P for c in range(NCORES)))
         for w in range(WPC)]
    Tsum = sum(T)                  # gather tiles
    Toh = [t + 1 for t in T]       # +1 self block per window
    Tohsum = sum(Toh)
    CTOT = 8 * Tsum                # int16 index columns

    selfw = dinv * dinv  # self-loop weight per node

    per_core = []
    for c in range(NCORES):
        gidx = np.zeros((P, CTOT), np.int16)
        ohf = np.zeros((P, Tohsum * P), np.float32)
        goff = 0   # gather tile offset
        ooff = 0   # oh column-block offset
        for w in range(WPC):
            us, inv, nrm, e_doff = uniq[c, w]
            base = c * NPC + w * P
            nreal = max(0, min(base + P, N) - base)
            # self block: diagonal of selfw * OS
            if nreal > 0:
                rr = np.arange(nreal)
                ohf[rr, ooff * P + rr] = selfw[base + rr] * OS
            # gather index columns (same node ids serve x and table2: the
            # single full AllGather keeps table2 rows in node order)
            cap = T[w] * P
            srcs = np.zeros(cap, np.int64)
            srcs[:len(us)] = us
            colbase = 8 * goff
            ii = np.arange(cap)
            gidx[ii % 16, colbase + ii // 16] = srcs.astype(np.int16)
            # one-hot: edge e -> oh[slot%128, blk(e)*P + doff]
            if len(nrm) > 0:
                cols = (ooff + 1 + inv // P) * P + e_doff
                np.add.at(ohf, (inv % P, cols), nrm * OS)
            goff += T[w]
            ooff += Toh[w]
        gidx[16:] = np.tile(gidx[:16], (7, 1))

        # tail masks [P, 8 * WPC]
        tmask = np.zeros((P, 8 * WPC), NPBF)
        for w in range(WPC):
            base = c * NPC + w * P
            nodes = base + np.arange(P)
            real = nodes < N
            for b2 in range(MAX_LIG):
                sel = real & (nodes >= b2 * APL) & (nodes < (b2 + 1) * APL)
                tmask[sel, 8 * w + b2] = 1.0
            for i in range(MAX_LIG):
                sel = nodes == pred_pos[i]
                tmask[sel, 8 * w + 3 + i] = 1.0
        per_core.append(dict(gidx=gidx, oh=_q8(ohf), tmask=tmask))

    x_pad = np.zeros((NPAD, NODE_D), NPF8)
    x_pad[:N] = _q8(np.asarray(x, np.float32) * XS)

    meta = dict(T=T, Tsum=Tsum, Toh=Toh, Tohsum=Tohsum, CTOT=CTOT)
    return meta, per_core, x_pad


def build_in_maps(inputs):
    meta, per_core, x_pad = _prep(
        np.asarray(inputs["x"], np.float32), np.asarray(inputs["edge_index"]),
        np.asarray(inputs["pred_pos"]))
    b1T = np.asarray(inputs["g1_b"], np.float32).reshape(HID // P, P).T.copy()
    b2T = (np.asarray(inputs["g2_b"], np.float32) * (T2S * OS)
           ).reshape(HID // P, P).T.copy()
    blp_rep = np.tile(np.asarray(inputs["lp_b"], np.float32)[None, :], (P, 1))
    g1 = (np.asarray(inputs["g1_w"], np.float32) / (XS * OS)).astype(NPBF)
    g2 = (np.asarray(inputs["g2_w"], np.float32) * T2S).astype(NPBF)
    lp = (np.asarray(inputs["lp_w"], np.float32) / (T2S * OS)).astype(NPBF)
    in_maps = []
    for c in range(NCORES):
        pc = per_core[c]
        in_maps.append(dict(
            x_f8=x_pad, xself=x_pad[c * NPC:(c + 1) * NPC].copy(),
            gidx=pc["gidx"], oh=pc["oh"],
            tmask=pc["tmask"], g1_w=g1, g2_w=g2, lp_w=lp,
            b1T=b1T, b2T=b2T, blp_rep=blp_rep,
        ))
    return meta, in_maps


# ----------------------------------------------------------------------------
# Device program
# ----------------------------------------------------------------------------

def _build(meta, reps=1):
    T = meta["T"]
    Tsum = meta["Tsum"]
    Toh = meta["Toh"]
    Tohsum = meta["Tohsum"]
    CTOT = meta["CTOT"]

    nc = bacc.Bacc("TRN2", target_bir_lowering=False, debug=False,
                   num_devices=NCORES, num_swdge_queues=NQ)

    # inputs
    d_x8 = nc.declare_dram_parameter("x_f8", [NPAD, NODE_D], F8, isOutput=False)
    d_xself = nc.declare_dram_parameter("xself", [NPC, NODE_D], F8, isOutput=False)
    d_gidx = nc.declare_dram_parameter("gidx", [P, CTOT], I16, isOutput=False)
    d_oh = nc.declare_dram_parameter("oh", [P, Tohsum * P], F8, isOutput=False)
    d_tmask = nc.declare_dram_parameter("tmask", [P, 8 * WPC], BF, isOutput=False)
    d_g1 = nc.declare_dram_parameter("g1_w", [NODE_D, HID], BF, isOutput=False)
    d_g2 = nc.declare_dram_parameter("g2_w", [HID, HID], BF, isOutput=False)
    d_lp = nc.declare_dram_parameter("lp_w", [HID, HID], BF, isOutput=False)
    d_b1T = nc.declare_dram_parameter("b1T", [P, HID // P], FP, isOutput=False)
    d_b2T = nc.declare_dram_parameter("b2T", [P, HID // P], FP, isOutput=False)
    d_blp = nc.declare_dram_parameter("blp_rep", [P, HID], FP, isOutput=False)
    # output
    d_tail = nc.declare_dram_parameter("out_tail", [8, HID], FP, isOutput=True)

    # internal dram (triple-buffered by rep parity so the exchange of rep r
    # never conflicts with older reps' layer-2 reads, two reps behind)
    local_rows = [nc.dram_tensor(f"local_rows{i}", [NPC, HID], F8)
                  for i in range(3)]
    table2 = [nc.dram_tensor(f"table2_{i}", [NPAD, HID], F8,
                             addr_space="Shared") for i in range(3)]

    mm = mybir.AluOpType
    act = mybir.ActivationFunctionType
    KC1 = NODE_D // P   # 2 feature chunks in layer-1 scatter
    KC2 = HID // P      # 4 chunks in layer-2 / dense

    with tile.TileContext(nc) as tc:
        with (
            tc.tile_pool(name="const", bufs=1) as cpool,
            tc.tile_pool(name="work", bufs=4) as wpool,
            tc.tile_pool(name="persist", bufs=3) as ppool,
            tc.tile_pool(name="hT", bufs=4) as hTpool,
            tc.tile_pool(name="ps_sc", bufs=3, space="PSUM") as ps_sc,
            tc.tile_pool(name="ps_mm", bufs=3, space="PSUM") as ps_mm,
            tc.tile_pool(name="ps_tail", bufs=1, space="PSUM") as ps_tail,
        ):
            # ---- constants in ----
            t_idx = cpool.tile([P, CTOT], I16)
            nc.sync.dma_start(t_idx[:], d_gidx[:])
            t_oh = cpool.tile([P, Tohsum * P], F8)
            nc.sync.dma_start(t_oh[:], d_oh[:])
            t_tmask = cpool.tile([P, 8 * WPC], BF)
            nc.sync.dma_start(t_tmask[:], d_tmask[:])
            t_g1 = cpool.tile([P, KC1, HID], BF)
            nc.sync.dma_start(t_g1[:], d_g1.rearrange("(a p) n -> p a n", p=P))
            t_g2 = cpool.tile([P, KC2, HID], BF)
            nc.sync.dma_start(t_g2[:], d_g2.rearrange("(a p) n -> p a n", p=P))
            t_lp = cpool.tile([P, KC2, HID], BF)
            nc.sync.dma_start(t_lp[:], d_lp.rearrange("(a p) n -> p a n", p=P))
            t_b1T = cpool.tile([P, HID // P], FP)
            nc.sync.dma_start(t_b1T[:], d_b1T[:])
            t_b2T = cpool.tile([P, HID // P], FP)
            nc.sync.dma_start(t_b2T[:], d_b2T[:])
            t_blp = cpool.tile([P, HID], FP)
            nc.sync.dma_start(t_blp[:], d_blp[:])

            def OH(col):
                return t_oh[:, col * P:(col + 1) * P]

            def OH2(col):
                return t_oh[:, col * P:(col + 2) * P].rearrange(
                    "p (j n) -> p j n", j=2)

            DR = mybir.MatmulPerfMode.DoubleRow

            def oh_scatter(out_ap, gt, fc, c0, Tn, first_start):
                # accumulate Tn gathered fp8 tiles against OH blocks
                # c0..c0+Tn-1, pairing tiles via fp8 DoubleRow (2 edge-tiles
                # contracted per matmul).
                t = 0
                first = first_start
                while t < Tn:
                    if t + 1 < Tn:
                        nc.tensor.matmul(
                            out_ap, gt[:, t:t + 2, fc * P:(fc + 1) * P],
                            OH2(c0 + t), start=first, stop=(t + 2 == Tn),
                            perf_mode=DR)
                        t += 2
                    else:
                        nc.tensor.matmul(
                            out_ap, gt[:, t, fc * P:(fc + 1) * P],
                            OH(c0 + t), start=first, stop=True)
                        t += 1
                    first = False

            qn = [0]

            def gathers(out_tile, table_ap, idx_tile, gcol, Tw, elem):
                # out_tile[:, t, :] = table[idx[gcol + t*128 + p]] tiles
                for t0 in range(0, Tw, GCHUNK):
                    t1 = min(t0 + GCHUNK, Tw)
                    nc.gpsimd.dma_gather(
                        out_ap=out_tile[:, t0:t1, :],
                        in_ap=table_ap,
                        idxs_ap=idx_tile[:, 8 * (gcol + t0): 8 * (gcol + t1)],
                        num_idxs=(t1 - t0) * P,
                        num_idxs_reg=(t1 - t0) * P,
                        elem_size=elem,
                        queue_num=qn[0] % NQ)
                    qn[0] += 1

            # self-row x tiles: core-local 1280 rows as [P, WPC, NODE_D]
            t_xs = cpool.tile([P, WPC, NODE_D], F8)
            nc.sync.dma_start(
                t_xs[:], d_xself.rearrange("(w p) n -> p w n", p=P))

            g1ctx = tc.tile_pool(name="g1pool", bufs=6)
            g1pool = g1ctx.__enter__()
            g2ctx = tc.tile_pool(name="g2pool", bufs=6)
            g2pool = g2ctx.__enter__()

            GOFF = np.cumsum([0] + T)     # gather tile offset per window
            OOFF = np.cumsum([0] + Toh)   # oh column-block offset per window

            def emit_l1_window(w, t2keep, lrows):
                Tw = T[w]
                goff, ooff = int(GOFF[w]), int(OOFF[w])
                g1t = g1pool.tile([P, Tw, NODE_D], F8, tag="gather1")
                gathers(g1t, d_x8[:], t_idx, goff, Tw, NODE_D)
                # aggT[k, d] = sum_e x[s_e, k] * norm_e [dst off d] (x64)
                psT = ps_sc.tile([P, NODE_D], FP, tag="psc")
                for fc in range(KC1):
                    nc.tensor.matmul(
                        psT[:, fc * P:(fc + 1) * P],
                        t_xs[:, w, fc * P:(fc + 1) * P],
                        OH(ooff), start=True, stop=False)
                    oh_scatter(psT[:, fc * P:(fc + 1) * P], g1t, fc,
                               ooff + 1, Tw, False)
                aggT = wpool.tile([P, NODE_D], BF, tag="aggT")
                nc.vector.tensor_copy(aggT[:], psT[:])
                # hT[j, d] = relu(sum_k (W1/64)[k, j] aggT[k, d] + b1)
                phT = ps_mm.tile([P, HID], FP, tag="pmm")
                for jc in range(KC2):
                    for fc in range(KC1):
                        nc.tensor.matmul(
                            phT[:, jc * P:(jc + 1) * P],
                            t_g1[:, fc, jc * P:(jc + 1) * P],
                            aggT[:, fc * P:(fc + 1) * P],
                            start=(fc == 0), stop=(fc == KC1 - 1))
                hT = hTpool.tile([P, HID], BF, tag="hT")
                for jc in range(KC2):
                    nc.scalar.activation(
                        hT[:, jc * P:(jc + 1) * P],
                        phT[:, jc * P:(jc + 1) * P],
                        act.Relu, bias=t_b1T[:, jc:jc + 1])
                # t2[d, m]*16 = sum_j h[d, j] (16*g2)[j, m] -> fp8 rows
                pt2 = ps_mm.tile([P, HID], FP, tag="pmm")
                for jc in range(KC2):
                    nc.tensor.matmul(pt2[:], hT[:, jc * P:(jc + 1) * P],
                                     t_g2[:, jc, :],
                                     start=(jc == 0), stop=(jc == KC2 - 1))
                nc.vector.tensor_copy(t2keep[:, w, :], pt2[:])
                nc.sync.dma_start(lrows[w * P:(w + 1) * P, :],
                                  t2keep[:, w, :])

            def emit_l2(st):
                t2keep = st["t2keep"]
                tab = st["table2"]
                ptail = ps_tail.tile([8, HID], FP)
                for w in range(WPC):
                    goff, ooff = int(GOFF[w]), int(OOFF[w])
                    g2t = g2pool.tile([P, T[w], HID], F8, tag="gather2")
                    gathers(g2t, tab[:], t_idx, goff, T[w], HID)
                    ps2 = ps_sc.tile([P, HID], FP, tag="psc")
                    for fc in range(KC2):
                        nc.tensor.matmul(
                            ps2[:, fc * P:(fc + 1) * P],
                            t2keep[:, w, fc * P:(fc + 1) * P],
                            OH(ooff), start=True, stop=False)
                        oh_scatter(ps2[:, fc * P:(fc + 1) * P], g2t, fc,
                                   ooff + 1, T[w], False)
                    # h2T*64 = relu(agg2*64 + 64*b2)
                    h2T = hTpool.tile([P, HID], BF, tag="hT")
                    for fc in range(KC2):
                        nc.scalar.activation(
                            h2T[:, fc * P:(fc + 1) * P],
                            ps2[:, fc * P:(fc + 1) * P],
                            act.Relu, bias=t_b2T[:, fc:fc + 1])
                    ph1 = ps_mm.tile([P, HID], FP, tag="pmm")
                    for jc in range(KC2):
                        nc.tensor.matmul(ph1[:], h2T[:, jc * P:(jc + 1) * P],
                                         t_lp[:, jc, :],
                                         start=(jc == 0), stop=(jc == KC2 - 1))
                    h1s = wpool.tile([P, HID], FP, tag="h1s")
                    nc.vector.tensor_add(h1s[:], ph1[:], t_blp[:])
                    h1 = wpool.tile([P, HID], BF, tag="h1")
                    nc.scalar.activation(h1[:], h1s[:], act.Relu)
                    nc.tensor.matmul(ptail[:], t_tmask[:, 8 * w:8 * w + 8],
                                     h1[:],
                                     start=(w == 0), stop=(w == WPC - 1))
                t_tail = wpool.tile([8, HID], FP, tag="tailout")
                nc.vector.tensor_copy(t_tail[:], ptail[:])
                nc.sync.dma_start(d_tail[:], t_tail[:])

            # Software pipeline across reps: one full AllGather per rep,
            # whose latency is hidden by ~two reps of other work (layer 2
            # trails its own rep by two). table2/local_rows rotate by rep%3.
            pending = []
            for rep_i in range(reps):
                pr = rep_i % 3
                t2keep = ppool.tile([P, WPC, HID], F8, tag="t2keep")
                st = {"t2keep": t2keep, "table2": table2[pr]}
                for w in range(WPC):
                    emit_l1_window(w, t2keep, local_rows[pr])
                nc.gpsimd.collective_compute(
                    "AllGather", mm.bypass,
                    replica_groups=[list(range(NCORES))],
                    ins=[local_rows[pr][:, :]], outs=[table2[pr][:, :]])
                pending.append(st)
                if len(pending) > 2:
                    emit_l2(pending.pop(0))
            for st in pending:
                emit_l2(st)
            g2ctx.__exit__(None, None, None)
            g1ctx.__exit__(None, None, None)

    nc.compile()
    return nc


# ----------------------------------------------------------------------------
# Host tail (metal branch + gates + 4-node TransformerConv + MLP head)
# ----------------------------------------------------------------------------

def _host_tail(tail, pred_pos, metal_id, metal_emb_table, mp_w, mp_b,
               gate_w1, gate_b1, gate_w2, gate_b2,
               tq_w, tq_b, tk_w, tk_b, tv_w, tv_b, tskip_w, tskip_b,
               pr_w1, pr_b1, pr_w2, pr_b2):
    f = np.float32
    pred_pos = np.asarray(pred_pos, np.int64)
    blocksum = tail[:3].astype(f)
    predrow = tail[3:6].astype(f)
    HEADS, HD = 8, HID // 8

    backbones = []
    for i in range(MAX_LIG):
        b = int(pred_pos[i]) // APL
        backbones.append((blocksum[b] - predrow[i]) / f(APL - 1))

    metal_node = np.maximum(
        np.asarray(metal_emb_table, f)[np.asarray(metal_id, np.int64)] @
        np.asarray(mp_w, f) + np.asarray(mp_b, f), 0)

    def tconv(hm, es, ed):
        n = hm.shape[0]
        q = (hm @ np.asarray(tq_w, f) + np.asarray(tq_b, f)).reshape(n, HEADS, HD)
        k = (hm @ np.asarray(tk_w, f) + np.asarray(tk_b, f)).reshape(n, HEADS, HD)
        v = (hm @ np.asarray(tv_w, f) + np.asarray(tv_b, f)).reshape(n, HEADS, HD)
        kj = k[es]
        vj = v[es]
        alpha = (q[ed] * kj).sum(-1) / np.sqrt(f(HD))
        amax = np.full((n, HEADS), -np.inf, f)
        np.maximum.at(amax, ed, alpha)
        ae = np.exp(alpha - amax[ed])
        den = np.zeros((n, HEADS), f)
        np.add.at(den, ed, ae)
        att = ae / den[ed]
        out = np.zeros((n, HEADS, HD), f)
        np.add.at(out, ed, vj * att[:, :, None])
        return out.reshape(n, HID) + hm @ np.asarray(tskip_w, f) + np.asarray(tskip_b, f)

    preds = []
    for n_lig in range(MAX_LIG, 0, -1):
        rows = []
        for i in range(n_lig):
            hb = backbones[i]
            g = 1.0 / (1.0 + np.exp(-(np.tanh(hb @ np.asarray(gate_w1, f) +
                                              np.asarray(gate_b1, f)) @
                                      np.asarray(gate_w2, f) +
                                      np.asarray(gate_b2, f))))
            rows.append(predrow[i] + g[0] * hb)
        hm = np.concatenate([metal_node, np.stack(rows)], 0).astype(f)
        es, ed = [], []
        for l in range(1, n_lig + 1):
            es += [0, l]
            ed += [l, 0]
        h3 = tconv(hm, np.array(es), np.array(ed))
        V = h3.mean(0)
        preds.append((V @ np.asarray(pr_w1, f) + np.asarray(pr_b1, f)) @
                     np.asarray(pr_w2, f) + np.asarray(pr_b2, f))
    return np.concatenate(preds).astype(np.float32)


# ----------------------------------------------------------------------------
# Entry point
# ----------------------------------------------------------------------------

def kernel(**inputs):
    meta, in_maps = build_in_maps(inputs)

    key = (meta["Tsum"], tuple(meta["T"]))
    nc = _RUN_CACHE.get(key)
    if nc is None:
        nc = _build(meta)
        _RUN_CACHE[key] = nc

    res = run_bass_kernel_spmd(nc, in_maps, list(range(NCORES)))
    tail = np.zeros((8, HID), np.float32)
    for c in range(NCORES):
        tail += res.results[c]["out_tail"]

    pred_pos = np.asarray(inputs["pred_pos"])
    return _host_tail(
        tail, pred_pos, inputs["metal_id"], inputs["metal_emb_table"],
        inputs["mp_w"], inputs["mp_b"],
        inputs["gate_w1"], inputs["gate_b1"], inputs["gate_w2"], inputs["gate_b2"],
        inputs["tq_w"], inputs["tq_b"], inputs["tk_w"], inputs["tk_b"],
        inputs["tv_w"], inputs["tv_b"], inputs["tskip_w"], inputs["tskip_b"],
        inputs["pr_w1"], inputs["pr_b1"], inputs["pr_w2"], inputs["pr_b2"])
